# revision 44
# baseline (speedup 1.0000x reference)
"""CategoryAttention (softmax over heads axis) on 8 Trainium2 cores.

Sharding (2x2x2): batch x q-half x k-half. Core c = b*4 + qi*2 + ki
handles batch b, query rows [qi*1024, qi*1024+1024), key rows
[ki*1024, ki*1024+1024). The softmax over the 16 heads is local to
each (q, k) position, and the AV contraction over k is a plain sum,
so each core produces a PARTIAL output (its k-half's contribution to
out = ctx @ Wo^T); the host sums the ki=0/ki=1 partials during the
gather. This removes the 4x-redundant K/V projections of a pure
q-sharded layout (every projection is now exactly 2x replicated).

Everything is bf16 on the PE (same rows/cycle as f32r, half the DMA
and SBUF), psum accumulation in f32. Engine assignment per k-tile:
  E:    PE 16 matmuls (C=64) into [P,2,512] psum tiles x3 bufs
  exp:  ACT psum->sbuf bf16 (scale=1/8)
  den:  DVE bf16 tree + f32 final add, reciprocal_approx_fast
  rb:   Pool f32->bf16 cast
  norm: DVE (8 heads) + Pool (8 heads) broadcast multiply
  AV:   PE 16 matmuls accumulating KC=2 k-tiles per psum bank pair
  ctx:  Pool copy (first round) / DVE add (later rounds), f32
Projection psum drains ride Pool; O-proj input is a bf16 cast of ctx.
"""

import numpy as np
from contextlib import ExitStack

import concourse.bass as bass
import concourse.tile as tile
from concourse import bacc, mybir
from concourse.bass_utils import run_bass_kernel_spmd
import ml_dtypes

F32 = mybir.dt.float32
BF16 = mybir.dt.bfloat16

N_CORES = 8
P = 128
D = 1024          # d_model
S = D // P        # 8 subtiles of the contraction dim
H = 16            # heads
HD = 64           # head dim
B = 2
L = 2048
NQ = 2            # q splits per batch
NK = 2            # k splits per batch
LQ = L // NQ      # 1024 query rows per core
LK = L // NK      # 1024 key rows per core
QC = 512          # q chunk (matmul N / psum bank width)
NQC = LQ // QC    # 2
KTS = 128         # k tile
NKT = LK // KTS   # 8
KC = 2            # k-tiles per AV psum accumulation chunk
DVE_HEADS = 8     # heads normalized on DVE; rest on Pool(gpsimd)
SCALE = 1.0 / np.sqrt(HD)

import os
BENCH_LOOP = int(os.environ.get("BENCH_LOOP", "1"))


def _build(has_bias):
    nc = bacc.Bacc("TRN2", target_bir_lowering=False, debug=False, num_devices=1)

    def din(name, shape, dt=BF16):
        return nc.dram_tensor(name, shape, dt, kind="ExternalInput").ap()

    qT_d = din("qT", (P, 2 * S * QC))          # 2 chunks of 512 q cols
    kT_d = din("kT", (P, 2 * S * QC))          # 2 chunks of 512 k cols
    vT_d = din("vT", (P, 2 * S * QC))
    wq_d = din("wq", (P, 2 * S * QC))          # 2 halves of 512 out cols
    wk_d = din("wk", (P, 2 * S * QC))
    wv_d = din("wv", (P, 2 * S * QC))
    wo_d = din("wo", (P, 2 * S * QC))
    bias_d = {}
    for nm in ("bq", "bk", "bv", "bo"):
        if has_bias[nm]:
            bias_d[nm] = din(nm, (1, D), F32)
    outT_d = nc.dram_tensor("outT", (P, S * LQ), F32, kind="ExternalOutput").ap()

    qT_ap = qT_d.rearrange("p (c s q) -> p c s q", c=2, s=S)
    kT_ap = kT_d.rearrange("p (c s k) -> p c s k", c=2, s=S)
    vT_ap = vT_d.rearrange("p (c s k) -> p c s k", c=2, s=S)
    wq_ap = wq_d.rearrange("p (h s o) -> p h s o", h=2, s=S)
    wk_ap = wk_d.rearrange("p (h s o) -> p h s o", h=2, s=S)
    wv_ap = wv_d.rearrange("p (h s o) -> p h s o", h=2, s=S)
    wo_ap = wo_d.rearrange("p (h s o) -> p h s o", h=2, s=S)
    outT_ap = outT_d.rearrange("p (j q) -> p j q", j=S)

    with tile.TileContext(nc) as tc, ExitStack() as ctx:
        if BENCH_LOOP > 1:
            ctx.enter_context(tc.For_i(0, BENCH_LOOP, 1))

        const_pool = ctx.enter_context(tc.tile_pool(name="const", bufs=1))
        qt_pool = ctx.enter_context(tc.tile_pool(name="QT", bufs=1))
        kt_pool = ctx.enter_context(tc.tile_pool(name="KT", bufs=1))
        v_pool = ctx.enter_context(tc.tile_pool(name="V", bufs=1))

        any_bias = any(has_bias.values())
        ones_t = None
        if any_bias:
            ones_t = const_pool.tile([1, QC], F32, tag="ones")
            nc.vector.memset(ones_t[:], 1.0)
        bias_t = {}
        for nm, d_ap in bias_d.items():
            t = const_pool.tile([1, D], F32, tag=f"bias_{nm}")
            nc.sync.dma_start(t[:], d_ap)
            bias_t[nm] = t

        QT_sb = qt_pool.tile([P, S, LQ], BF16)
        KT_sb = kt_pool.tile([P, S, LK], BF16)
        V_sb = v_pool.tile([P, NKT, D], BF16)

        def bias_mm(ps_t, bias_name, o0, n_sz, o_on_partitions):
            if o_on_partitions:
                nc.tensor.matmul(ps_t, lhsT=bias_t[bias_name][0:1, o0:o0 + P],
                                 rhs=ones_t[0:1, :n_sz], start=False, stop=True)
            else:
                nc.tensor.matmul(ps_t, lhsT=ones_t[0:1, 0:P],
                                 rhs=bias_t[bias_name][0:1, o0:o0 + n_sz],
                                 start=False, stop=True)

        # ---------------- pools (projection + attention share psum) -----
        spool = ctx.enter_context(tc.tile_pool(name="stream", bufs=1))
        attn_pool = ctx.enter_context(tc.tile_pool(name="attn", bufs=3))
        tree_pool = ctx.enter_context(tc.tile_pool(name="tree", bufs=2))
        den_pool = ctx.enter_context(tc.tile_pool(name="den", bufs=1))
        r_pool = ctx.enter_context(tc.tile_pool(name="r", bufs=1))
        rb_pool = ctx.enter_context(tc.tile_pool(name="rb", bufs=2))
        ctx_pool = ctx.enter_context(tc.tile_pool(name="ctx", bufs=1))
        inpool = ctx.enter_context(tc.tile_pool(name="inp", bufs=3))
        osb_pool = ctx.enter_context(tc.tile_pool(name="osb", bufs=2))
        e_psum = ctx.enter_context(tc.tile_pool(name="epsum", bufs=3, space="PSUM"))
        av_psum = ctx.enter_context(tc.tile_pool(name="avpsum", bufs=1, space="PSUM"))

        ctx_sb = ctx_pool.tile([P, S, LQ], BF16)

        def load_w(src_ap, tag, eng=None):
            t = spool.tile([P, 2, S, QC], BF16, tag=tag)
            (eng or nc.sync).dma_start(t[:], src_ap)
            return t

        def load_in(src_ap, eng=None):
            t = inpool.tile([P, S, QC], BF16, tag="in")
            (eng or nc.sync).dma_start(t[:], src_ap)
            return t

        def proj_pair(w_halves, in_t, out_view, bias_name, j2):
            """One [P,2,QC] psum group: output cols j2*256..j2*256+256."""
            ps = e_psum.tile([P, 2, QC], F32, tag="e")
            for jj in range(2):
                j = j2 * 2 + jj
                w_t = w_halves[j // 4]
                jl = j % 4
                for s in range(S):
                    nc.tensor.matmul(
                        ps[:, jj, :],
                        lhsT=w_t[:, s, jl * P:(jl + 1) * P],
                        rhs=in_t[:, s, :],
                        start=(s == 0),
                        stop=(s == S - 1 and bias_name is None),
                    )
                if bias_name is not None:
                    bias_mm(ps[:, jj, :], bias_name, j * P, QC, True)
            nc.scalar.copy(out_view[:, j2 * 2:j2 * 2 + 2, :], ps[:])

        def v_group(vin, kt4, kt, wv_h):
            """Project V rows for one k-tile into V_sb[:, kt, :]."""
            ps = e_psum.tile([P, 2, QC], F32, tag="e")
            for t in range(2):
                for s in range(S):
                    nc.tensor.matmul(
                        ps[:, t, :],
                        lhsT=vin[:, s, kt4 * P:(kt4 + 1) * P],
                        rhs=wv_h[t][:, s, :],
                        start=(s == 0),
                        stop=(s == S - 1 and not has_bias["bv"]),
                    )
                if has_bias["bv"]:
                    bias_mm(ps[:, t, :], "bv", t * QC, QC, False)
            nc.scalar.copy(V_sb[:, kt, :],
                           ps[:].rearrange("p a b -> p (a b)"))

        def softmax_kt(kt, qc):
            """Energy (16 heads) -> exp -> den -> normalized attn tile."""
            q0 = qc * QC
            attn_t = attn_pool.tile([P, H, QC], BF16, tag="attn")
            for g in range(8):  # 2 heads per psum tile, one bank per head
                eps = e_psum.tile([P, 2, QC], F32, tag="e")
                for hh in range(2):
                    h = g * 2 + hh
                    j2, p0 = h // 2, HD * (h % 2)
                    nc.tensor.matmul(
                        eps[:, hh, :],
                        lhsT=KT_sb[p0:p0 + HD, j2, kt * KTS:(kt + 1) * KTS],
                        rhs=QT_sb[p0:p0 + HD, j2, q0:q0 + QC],
                        start=True,
                        stop=True,
                    )
                nc.scalar.activation(attn_t[:, g * 2:(g + 1) * 2, :], eps[:],
                                     mybir.ActivationFunctionType.Exp,
                                     scale=float(SCALE))
            # den = sum over heads (bf16 tree on DVE; final add f32)
            t1 = tree_pool.tile([P, 4, QC], BF16, tag="t1")
            with nc.allow_low_precision(reason="bf16 head-sum tree"):
                nc.vector.tensor_add(t1[:], attn_t[:, 0:4, :], attn_t[:, 4:8, :])
                nc.vector.tensor_add(t1[:], t1[:], attn_t[:, 8:12, :])
                nc.vector.tensor_add(t1[:], t1[:], attn_t[:, 12:16, :])
                nc.vector.tensor_add(t1[:, 0:2, :], t1[:, 0:2, :], t1[:, 2:4, :])
            den = den_pool.tile([P, QC], F32, tag="den")
            nc.vector.tensor_add(den[:], t1[:, 0, :], t1[:, 1, :])
            r32 = r_pool.tile([P, QC], F32, tag="r")
            nc.vector.reciprocal_approx_fast(r32[:], den[:])
            rb = rb_pool.tile([P, QC], BF16, tag="rb")
            nc.vector.tensor_copy(rb[:], r32[:])
            nc.vector.tensor_mul(
                attn_t[:], attn_t[:],
                rb[:, None, :].to_broadcast((P, H, QC)))
            return attn_t

        def av_group(u, qc, c0, attn_list, first, pool=None):
            """One avp tile: heads 4u..4u+3, one q chunk, over KC k-tiles."""
            avp = (pool or av_psum).tile([P, 2, QC], F32,
                                         tag="av" if pool is None else "e")
            for ci in range(KC):
                kt = c0 + ci
                for hh in range(4):
                    h = 4 * u + hh
                    i, p0 = hh // 2, HD * (hh % 2)
                    nc.tensor.matmul(
                        avp[p0:p0 + HD, i, :],
                        lhsT=V_sb[:, kt, h * HD:(h + 1) * HD],
                        rhs=attn_list[ci][:, h, :],
                        start=(ci == 0),
                        stop=(ci == KC - 1),
                    )
            view = ctx_sb[:, 2 * u:2 * u + 2, qc * QC:(qc + 1) * QC]
            if first:
                nc.vector.tensor_copy(view, avp[:, :, :])
            else:
                with nc.allow_low_precision(reason="bf16 ctx accumulate"):
                    nc.vector.tensor_add(view, view, avp[:, :, :])

        def out_proj(qc, wo_h):
            """O-proj directly on this q-chunk's bf16 partial ctx."""
            ctxv = ctx_sb[:, :, qc * QC:(qc + 1) * QC]
            for j4 in range(2):
                woh = wo_h[j4]
                for j2 in range(2):
                    po = e_psum.tile([P, 2, QC], F32, tag="e")
                    for jj in range(2):
                        j = j4 * 4 + j2 * 2 + jj
                        jl = j2 * 2 + jj
                        for s in range(S):
                            nc.tensor.matmul(
                                po[:, jj, :],
                                lhsT=woh[:, s, jl * P:(jl + 1) * P],
                                rhs=ctxv[:, s, :],
                                start=(s == 0),
                                stop=(s == S - 1 and not has_bias["bo"]),
                            )
                        if has_bias["bo"]:
                            bias_mm(po[:, jj, :], "bo", j * P, QC, True)
                    j0 = j4 * 4 + j2 * 2
                    osb = osb_pool.tile([P, 2, QC], F32, tag="osb")
                    nc.scalar.copy(osb[:], po[:])
                    nc.sync.dma_start(
                        outT_ap[:, j0:j0 + 2, qc * QC:(qc + 1) * QC],
                        osb[:])

        # ---------------- schedule ----------------
        # wq halves as separate tiles: the first Q-proj group only needs
        # half 0 + q0, so compute starts after 16KB of DMA instead of 24KB
        def load_half(src_ap, tag):
            t = spool.tile([P, S, QC], BF16, tag=tag)
            nc.sync.dma_start(t[:], src_ap)
            return t

        wq0 = load_half(wq_ap[:, 0], "wa0")
        q0 = load_in(qT_ap[:, 0])
        wq1 = load_half(wq_ap[:, 1], "wa1")
        q1 = load_in(qT_ap[:, 1])
        wk_t = load_w(wk_ap, "wb")
        k0 = load_in(kT_ap[:, 0])
        wq_h = [wq0, wq1]
        wk_h = [wk_t[:, h] for h in range(2)]

        bq = "bq" if has_bias["bq"] else None
        bk = "bk" if has_bias["bk"] else None
        for qn, qt in enumerate((q0, q1)):
            for j2 in range(4):
                proj_pair(wq_h, qt,
                          QT_sb[:, :, qn * QC:(qn + 1) * QC], bq, j2)
        k1 = load_in(kT_ap[:, 1])
        wv_t = load_w(wv_ap, "wc")
        v0 = load_in(vT_ap[:, 0])
        wv_h = [wv_t[:, h] for h in range(2)]
        for j2 in range(4):
            proj_pair(wk_h, k0, KT_sb[:, :, 0:QC], bk, j2)
        v1 = load_in(vT_ap[:, 1])
        # wo reuses the wq half-buffers after Q-proj
        wo_h = [load_half(wo_ap[:, 0], "wa0"), load_half(wo_ap[:, 1], "wa1")]

        for qc in range(NQC):
            prev = None  # (c0, attn_list)
            for ch in range(NKT // KC):
                c0 = ch * KC
                cur = []
                for ci in range(KC):
                    kt = c0 + ci
                    # issue one av_group before and one after each softmax
                    # so E-matmuls hide the shared av-psum drain latency
                    if prev is not None:
                        av_group(2 * ci, qc, prev[0], prev[1], prev[0] == 0)
                    cur.append(softmax_kt(kt, qc))
                    if qc == 0:
                        # interleave the second K chunk and all V k-tiles
                        # into the first attention pass
                        if kt < 4:
                            proj_pair(wk_h, k1,
                                      KT_sb[:, :, QC:2 * QC], bk, kt)
                        v_group(v0 if kt < 4 else v1, kt % 4, kt, wv_h)
                    if prev is not None:
                        av_group(2 * ci + 1, qc, prev[0], prev[1],
                                 prev[0] == 0)
                prev = (c0, cur)
            for u in range(4):
                # alternate psum pools so drains of consecutive groups overlap
                av_group(u, qc, prev[0], prev[1], False,
                         pool=e_psum if u % 2 else None)
            out_proj(qc, wo_h)

    nc.compile()
    return nc


_cache = {}


def _get_program(has_bias):
    key = (BENCH_LOOP, tuple(sorted(has_bias.items())))
    if key not in _cache:
        _cache[key] = _build(has_bias)
    return _cache[key]


def _chunked(x, width=QC):
    """[D, N] f32 -> bf16 [P, N//width, S, width] per-chunk contiguous."""
    n = x.shape[1]
    nch = n // width
    y = x.reshape(S, P, nch, width).transpose(1, 2, 0, 3)
    return np.ascontiguousarray(
        y.reshape(P, nch * S * width).astype(ml_dtypes.bfloat16))


def prepare_inputs(query, key, value, Wq_w, Wq_b, Wk_w, Wk_b, Wv_w, Wv_b,
                   Wo_w, Wo_b):
    query = np.asarray(query, dtype=np.float32)
    key = np.asarray(key, dtype=np.float32)
    value = np.asarray(value, dtype=np.float32)
    w = {
        "wq": _chunked(np.ascontiguousarray(np.asarray(Wq_w, np.float32).T)),
        "wk": _chunked(np.ascontiguousarray(np.asarray(Wk_w, np.float32).T)),
        "wv": _chunked(np.ascontiguousarray(np.asarray(Wv_w, np.float32).T)),
        "wo": _chunked(np.ascontiguousarray(np.asarray(Wo_w, np.float32).T)),
    }
    biases = {"bq": np.asarray(Wq_b, np.float32), "bk": np.asarray(Wk_b, np.float32),
              "bv": np.asarray(Wv_b, np.float32), "bo": np.asarray(Wo_b, np.float32)}
    has_bias = {nm: bool(np.any(b)) for nm, b in biases.items()}

    qT = [[_chunked(np.ascontiguousarray(query[b, qi * LQ:(qi + 1) * LQ].T))
           for qi in range(NQ)] for b in range(B)]
    kT = [[_chunked(np.ascontiguousarray(key[b, ki * LK:(ki + 1) * LK].T))
           for ki in range(NK)] for b in range(B)]
    vT = [[_chunked(np.ascontiguousarray(value[b, ki * LK:(ki + 1) * LK].T))
           for ki in range(NK)] for b in range(B)]

    in_maps = []
    for c in range(N_CORES):
        b, qi, ki = c // 4, (c % 4) // 2, c % 2
        m = {
            "qT": qT[b][qi],
            "kT": kT[b][ki],
            "vT": vT[b][ki],
            **w,
        }
        for nm, hb in has_bias.items():
            if hb:
                m[nm] = biases[nm].reshape(1, D)
        in_maps.append(m)
    return in_maps, has_bias


def gather_output(results):
    out = np.empty((B, L, D), dtype=np.float32)
    for b in range(B):
        for qi in range(NQ):
            acc = None
            for ki in range(NK):
                c = b * 4 + qi * 2 + ki
                part = results[c]["outT"]
                acc = part if acc is None else acc + part
            oT = acc.reshape(P, S, LQ).transpose(1, 0, 2).reshape(D, LQ)
            out[b, qi * LQ:(qi + 1) * LQ, :] = oT.T
    return out


def kernel(**inputs) -> np.ndarray:
    in_maps, has_bias = prepare_inputs(**inputs)
    nc = _get_program(has_bias)
    res = run_bass_kernel_spmd(nc, in_maps, list(range(N_CORES)))
    return gather_output(res.results)


# revision 45
# speedup vs baseline: 1.0109x; 1.0109x over previous
"""CategoryAttention (softmax over heads axis) on 8 Trainium2 cores.

Sharding (2x2x2): batch x q-half x k-half. Core c = b*4 + qi*2 + ki
handles batch b, query rows [qi*1024, qi*1024+1024), key rows
[ki*1024, ki*1024+1024). The softmax over the 16 heads is local to
each (q, k) position, and the AV contraction over k is a plain sum,
so each core produces a PARTIAL output (its k-half's contribution to
out = ctx @ Wo^T); the host sums the ki=0/ki=1 partials during the
gather. This removes the 4x-redundant K/V projections of a pure
q-sharded layout (every projection is now exactly 2x replicated).

Everything is bf16 on the PE (same rows/cycle as f32r, half the DMA
and SBUF), psum accumulation in f32. Engine assignment per k-tile:
  E:    PE 16 matmuls (C=64) into [P,2,512] psum tiles x3 bufs
  exp:  ACT psum->sbuf bf16 (scale=1/8)
  den:  DVE bf16 tree + f32 final add, reciprocal_approx_fast
  rb:   Pool f32->bf16 cast
  norm: DVE (8 heads) + Pool (8 heads) broadcast multiply
  AV:   PE 16 matmuls accumulating KC=2 k-tiles per psum bank pair
  ctx:  Pool copy (first round) / DVE add (later rounds), f32
Projection psum drains ride Pool; O-proj input is a bf16 cast of ctx.
"""

import numpy as np
from contextlib import ExitStack

import concourse.bass as bass
import concourse.tile as tile
from concourse import bacc, mybir
from concourse.bass_utils import run_bass_kernel_spmd
import ml_dtypes

F32 = mybir.dt.float32
BF16 = mybir.dt.bfloat16

N_CORES = 8
P = 128
D = 1024          # d_model
S = D // P        # 8 subtiles of the contraction dim
H = 16            # heads
HD = 64           # head dim
B = 2
L = 2048
NQ = 2            # q splits per batch
NK = 2            # k splits per batch
LQ = L // NQ      # 1024 query rows per core
LK = L // NK      # 1024 key rows per core
QC = 512          # q chunk (matmul N / psum bank width)
NQC = LQ // QC    # 2
KTS = 128         # k tile
NKT = LK // KTS   # 8
KC = 2            # k-tiles per AV psum accumulation chunk
DVE_HEADS = 8     # heads normalized on DVE; rest on Pool(gpsimd)
SCALE = 1.0 / np.sqrt(HD)

import os
BENCH_LOOP = int(os.environ.get("BENCH_LOOP", "1"))


def _build(has_bias):
    nc = bacc.Bacc("TRN2", target_bir_lowering=False, debug=False, num_devices=1)

    def din(name, shape, dt=BF16):
        return nc.dram_tensor(name, shape, dt, kind="ExternalInput").ap()

    qT_d = din("qT", (P, 2 * S * QC))          # 2 chunks of 512 q cols
    kT_d = din("kT", (P, 2 * S * QC))          # 2 chunks of 512 k cols
    vT_d = din("vT", (P, 2 * S * QC))
    wq_d = din("wq", (P, 2 * S * QC))          # 2 halves of 512 out cols
    wk_d = din("wk", (P, 2 * S * QC))
    wv_d = din("wv", (P, 2 * S * QC))
    wo_d = din("wo", (P, 2 * S * QC))
    bias_d = {}
    for nm in ("bq", "bk", "bv", "bo"):
        if has_bias[nm]:
            bias_d[nm] = din(nm, (1, D), F32)
    outT_d = nc.dram_tensor("outT", (P, S * LQ), F32, kind="ExternalOutput").ap()

    qT_ap = qT_d.rearrange("p (c s q) -> p c s q", c=2, s=S)
    kT_ap = kT_d.rearrange("p (c s k) -> p c s k", c=2, s=S)
    vT_ap = vT_d.rearrange("p (c s k) -> p c s k", c=2, s=S)
    wq_ap = wq_d.rearrange("p (h s o) -> p h s o", h=2, s=S)
    wk_ap = wk_d.rearrange("p (h s o) -> p h s o", h=2, s=S)
    wv_ap = wv_d.rearrange("p (h s o) -> p h s o", h=2, s=S)
    wo_ap = wo_d.rearrange("p (h s o) -> p h s o", h=2, s=S)
    outT_ap = outT_d.rearrange("p (j q) -> p j q", j=S)

    with tile.TileContext(nc) as tc, ExitStack() as ctx:
        if BENCH_LOOP > 1:
            ctx.enter_context(tc.For_i(0, BENCH_LOOP, 1))

        const_pool = ctx.enter_context(tc.tile_pool(name="const", bufs=1))
        qt_pool = ctx.enter_context(tc.tile_pool(name="QT", bufs=1))
        kt_pool = ctx.enter_context(tc.tile_pool(name="KT", bufs=1))
        v_pool = ctx.enter_context(tc.tile_pool(name="V", bufs=1))

        any_bias = any(has_bias.values())
        ones_t = None
        if any_bias:
            ones_t = const_pool.tile([1, QC], F32, tag="ones")
            nc.vector.memset(ones_t[:], 1.0)
        bias_t = {}
        for nm, d_ap in bias_d.items():
            t = const_pool.tile([1, D], F32, tag=f"bias_{nm}")
            nc.sync.dma_start(t[:], d_ap)
            bias_t[nm] = t

        QT_sb = qt_pool.tile([P, S, LQ], BF16)
        KT_sb = kt_pool.tile([P, S, LK], BF16)
        V_sb = v_pool.tile([P, NKT, D], BF16)

        def bias_mm(ps_t, bias_name, o0, n_sz, o_on_partitions):
            if o_on_partitions:
                nc.tensor.matmul(ps_t, lhsT=bias_t[bias_name][0:1, o0:o0 + P],
                                 rhs=ones_t[0:1, :n_sz], start=False, stop=True)
            else:
                nc.tensor.matmul(ps_t, lhsT=ones_t[0:1, 0:P],
                                 rhs=bias_t[bias_name][0:1, o0:o0 + n_sz],
                                 start=False, stop=True)

        # ---------------- pools (projection + attention share psum) -----
        spool = ctx.enter_context(tc.tile_pool(name="stream", bufs=1))
        attn_pool = ctx.enter_context(tc.tile_pool(name="attn", bufs=3))
        tree_pool = ctx.enter_context(tc.tile_pool(name="tree", bufs=2))
        den_pool = ctx.enter_context(tc.tile_pool(name="den", bufs=1))
        r_pool = ctx.enter_context(tc.tile_pool(name="r", bufs=1))
        rb_pool = ctx.enter_context(tc.tile_pool(name="rb", bufs=2))
        ctx_pool = ctx.enter_context(tc.tile_pool(name="ctx", bufs=1))
        inpool = ctx.enter_context(tc.tile_pool(name="inp", bufs=3))
        osb_pool = ctx.enter_context(tc.tile_pool(name="osb", bufs=2))
        e_psum = ctx.enter_context(tc.tile_pool(name="epsum", bufs=3, space="PSUM"))
        av_psum = ctx.enter_context(tc.tile_pool(name="avpsum", bufs=1, space="PSUM"))

        ctx_sb = ctx_pool.tile([P, S, LQ], BF16)

        def load_w(src_ap, tag, eng=None):
            t = spool.tile([P, 2, S, QC], BF16, tag=tag)
            (eng or nc.sync).dma_start(t[:], src_ap)
            return t

        def load_in(src_ap, eng=None):
            t = inpool.tile([P, S, QC], BF16, tag="in")
            (eng or nc.sync).dma_start(t[:], src_ap)
            return t

        def proj_pair(w_halves, in_t, out_view, bias_name, j2):
            """One [P,2,QC] psum group: output cols j2*256..j2*256+256."""
            ps = e_psum.tile([P, 2, QC], F32, tag="e")
            for jj in range(2):
                j = j2 * 2 + jj
                w_t = w_halves[j // 4]
                jl = j % 4
                for s in range(S):
                    nc.tensor.matmul(
                        ps[:, jj, :],
                        lhsT=w_t[:, s, jl * P:(jl + 1) * P],
                        rhs=in_t[:, s, :],
                        start=(s == 0),
                        stop=(s == S - 1 and bias_name is None),
                    )
                if bias_name is not None:
                    bias_mm(ps[:, jj, :], bias_name, j * P, QC, True)
            nc.scalar.copy(out_view[:, j2 * 2:j2 * 2 + 2, :], ps[:])

        def v_group(vin, kt4, kt, wv_h):
            """Project V rows for one k-tile into V_sb[:, kt, :]."""
            ps = e_psum.tile([P, 2, QC], F32, tag="e")
            for t in range(2):
                for s in range(S):
                    nc.tensor.matmul(
                        ps[:, t, :],
                        lhsT=vin[:, s, kt4 * P:(kt4 + 1) * P],
                        rhs=wv_h[t][:, s, :],
                        start=(s == 0),
                        stop=(s == S - 1 and not has_bias["bv"]),
                    )
                if has_bias["bv"]:
                    bias_mm(ps[:, t, :], "bv", t * QC, QC, False)
            nc.scalar.copy(V_sb[:, kt, :],
                           ps[:].rearrange("p a b -> p (a b)"))

        def softmax_kt(kt, qc):
            """Energy (16 heads) -> exp -> den -> normalized attn tile."""
            q0 = qc * QC
            attn_t = attn_pool.tile([P, H, QC], BF16, tag="attn")
            for g in range(8):  # 2 heads per psum tile, one bank per head
                eps = e_psum.tile([P, 2, QC], F32, tag="e")
                for hh in range(2):
                    h = g * 2 + hh
                    j2, p0 = h // 2, HD * (h % 2)
                    nc.tensor.matmul(
                        eps[:, hh, :],
                        lhsT=KT_sb[p0:p0 + HD, j2, kt * KTS:(kt + 1) * KTS],
                        rhs=QT_sb[p0:p0 + HD, j2, q0:q0 + QC],
                        start=True,
                        stop=True,
                    )
                nc.scalar.activation(attn_t[:, g * 2:(g + 1) * 2, :], eps[:],
                                     mybir.ActivationFunctionType.Exp,
                                     scale=float(SCALE))
            # den = sum over heads (bf16 tree on DVE; final add f32)
            t1 = tree_pool.tile([P, 4, QC], BF16, tag="t1")
            with nc.allow_low_precision(reason="bf16 head-sum tree"):
                nc.vector.tensor_add(t1[:], attn_t[:, 0:4, :], attn_t[:, 4:8, :])
                nc.vector.tensor_add(t1[:], t1[:], attn_t[:, 8:12, :])
                nc.vector.tensor_add(t1[:], t1[:], attn_t[:, 12:16, :])
                nc.vector.tensor_add(t1[:, 0:2, :], t1[:, 0:2, :], t1[:, 2:4, :])
            den = den_pool.tile([P, QC], F32, tag="den")
            nc.vector.tensor_add(den[:], t1[:, 0, :], t1[:, 1, :])
            r32 = r_pool.tile([P, QC], F32, tag="r")
            nc.vector.reciprocal_approx_fast(r32[:], den[:])
            rb = rb_pool.tile([P, QC], BF16, tag="rb")
            nc.vector.tensor_copy(rb[:], r32[:])
            nc.vector.tensor_mul(
                attn_t[:], attn_t[:],
                rb[:, None, :].to_broadcast((P, H, QC)))
            return attn_t

        def av_group(u, qc, c0, attn_list, first, pool=None):
            """One avp tile: heads 4u..4u+3, one q chunk, over KC k-tiles."""
            avp = (pool or av_psum).tile([P, 2, QC], F32,
                                         tag="av" if pool is None else "e")
            for ci in range(KC):
                kt = c0 + ci
                for hh in range(4):
                    h = 4 * u + hh
                    i, p0 = hh // 2, HD * (hh % 2)
                    nc.tensor.matmul(
                        avp[p0:p0 + HD, i, :],
                        lhsT=V_sb[:, kt, h * HD:(h + 1) * HD],
                        rhs=attn_list[ci][:, h, :],
                        start=(ci == 0),
                        stop=(ci == KC - 1),
                    )
            view = ctx_sb[:, 2 * u:2 * u + 2, qc * QC:(qc + 1) * QC]
            if first:
                nc.scalar.copy(view, avp[:, :, :])
            else:
                with nc.allow_low_precision(reason="bf16 ctx accumulate"):
                    nc.vector.tensor_add(view, view, avp[:, :, :])

        def out_proj(qc, wo_h):
            """O-proj directly on this q-chunk's bf16 partial ctx."""
            ctxv = ctx_sb[:, :, qc * QC:(qc + 1) * QC]
            for j4 in range(2):
                woh = wo_h[j4]
                for j2 in range(2):
                    po = e_psum.tile([P, 2, QC], F32, tag="e")
                    for jj in range(2):
                        j = j4 * 4 + j2 * 2 + jj
                        jl = j2 * 2 + jj
                        for s in range(S):
                            nc.tensor.matmul(
                                po[:, jj, :],
                                lhsT=woh[:, s, jl * P:(jl + 1) * P],
                                rhs=ctxv[:, s, :],
                                start=(s == 0),
                                stop=(s == S - 1 and not has_bias["bo"]),
                            )
                        if has_bias["bo"]:
                            bias_mm(po[:, jj, :], "bo", j * P, QC, True)
                    j0 = j4 * 4 + j2 * 2
                    osb = osb_pool.tile([P, 2, QC], F32, tag="osb")
                    nc.scalar.copy(osb[:], po[:])
                    nc.sync.dma_start(
                        outT_ap[:, j0:j0 + 2, qc * QC:(qc + 1) * QC],
                        osb[:])

        # ---------------- schedule ----------------
        # wq halves as separate tiles: the first Q-proj group only needs
        # half 0 + q0, so compute starts after 16KB of DMA instead of 24KB
        def load_half(src_ap, tag):
            t = spool.tile([P, S, QC], BF16, tag=tag)
            nc.sync.dma_start(t[:], src_ap)
            return t

        wq0 = load_half(wq_ap[:, 0], "wa0")
        q0 = load_in(qT_ap[:, 0])
        wq1 = load_half(wq_ap[:, 1], "wa1")
        q1 = load_in(qT_ap[:, 1])
        wk_t = load_w(wk_ap, "wb")
        k0 = load_in(kT_ap[:, 0])
        wq_h = [wq0, wq1]
        wk_h = [wk_t[:, h] for h in range(2)]

        bq = "bq" if has_bias["bq"] else None
        bk = "bk" if has_bias["bk"] else None
        for qn, qt in enumerate((q0, q1)):
            for j2 in range(4):
                proj_pair(wq_h, qt,
                          QT_sb[:, :, qn * QC:(qn + 1) * QC], bq, j2)
        k1 = load_in(kT_ap[:, 1])
        wv_t = load_w(wv_ap, "wc")
        v0 = load_in(vT_ap[:, 0])
        wv_h = [wv_t[:, h] for h in range(2)]
        for j2 in range(4):
            proj_pair(wk_h, k0, KT_sb[:, :, 0:QC], bk, j2)
        v1 = load_in(vT_ap[:, 1])
        # wo reuses the wq half-buffers after Q-proj
        wo_h = [load_half(wo_ap[:, 0], "wa0"), load_half(wo_ap[:, 1], "wa1")]

        for qc in range(NQC):
            prev = None  # (c0, attn_list)
            for ch in range(NKT // KC):
                c0 = ch * KC
                cur = []
                for ci in range(KC):
                    kt = c0 + ci
                    # issue one av_group before and one after each softmax
                    # so E-matmuls hide the shared av-psum drain latency
                    if prev is not None:
                        av_group(2 * ci, qc, prev[0], prev[1], prev[0] == 0)
                    cur.append(softmax_kt(kt, qc))
                    if qc == 0:
                        # interleave the second K chunk and all V k-tiles
                        # into the first attention pass
                        if kt < 4:
                            proj_pair(wk_h, k1,
                                      KT_sb[:, :, QC:2 * QC], bk, kt)
                        v_group(v0 if kt < 4 else v1, kt % 4, kt, wv_h)
                    if prev is not None:
                        av_group(2 * ci + 1, qc, prev[0], prev[1],
                                 prev[0] == 0)
                prev = (c0, cur)
            for u in range(4):
                # alternate psum pools so drains of consecutive groups overlap
                av_group(u, qc, prev[0], prev[1], False,
                         pool=e_psum if u % 2 else None)
            out_proj(qc, wo_h)

    nc.compile()
    return nc


_cache = {}


def _get_program(has_bias):
    key = (BENCH_LOOP, tuple(sorted(has_bias.items())))
    if key not in _cache:
        _cache[key] = _build(has_bias)
    return _cache[key]


def _chunked(x, width=QC):
    """[D, N] f32 -> bf16 [P, N//width, S, width] per-chunk contiguous."""
    n = x.shape[1]
    nch = n // width
    y = x.reshape(S, P, nch, width).transpose(1, 2, 0, 3)
    return np.ascontiguousarray(
        y.reshape(P, nch * S * width).astype(ml_dtypes.bfloat16))


def prepare_inputs(query, key, value, Wq_w, Wq_b, Wk_w, Wk_b, Wv_w, Wv_b,
                   Wo_w, Wo_b):
    query = np.asarray(query, dtype=np.float32)
    key = np.asarray(key, dtype=np.float32)
    value = np.asarray(value, dtype=np.float32)
    w = {
        "wq": _chunked(np.ascontiguousarray(np.asarray(Wq_w, np.float32).T)),
        "wk": _chunked(np.ascontiguousarray(np.asarray(Wk_w, np.float32).T)),
        "wv": _chunked(np.ascontiguousarray(np.asarray(Wv_w, np.float32).T)),
        "wo": _chunked(np.ascontiguousarray(np.asarray(Wo_w, np.float32).T)),
    }
    biases = {"bq": np.asarray(Wq_b, np.float32), "bk": np.asarray(Wk_b, np.float32),
              "bv": np.asarray(Wv_b, np.float32), "bo": np.asarray(Wo_b, np.float32)}
    has_bias = {nm: bool(np.any(b)) for nm, b in biases.items()}

    qT = [[_chunked(np.ascontiguousarray(query[b, qi * LQ:(qi + 1) * LQ].T))
           for qi in range(NQ)] for b in range(B)]
    kT = [[_chunked(np.ascontiguousarray(key[b, ki * LK:(ki + 1) * LK].T))
           for ki in range(NK)] for b in range(B)]
    vT = [[_chunked(np.ascontiguousarray(value[b, ki * LK:(ki + 1) * LK].T))
           for ki in range(NK)] for b in range(B)]

    in_maps = []
    for c in range(N_CORES):
        b, qi, ki = c // 4, (c % 4) // 2, c % 2
        m = {
            "qT": qT[b][qi],
            "kT": kT[b][ki],
            "vT": vT[b][ki],
            **w,
        }
        for nm, hb in has_bias.items():
            if hb:
                m[nm] = biases[nm].reshape(1, D)
        in_maps.append(m)
    return in_maps, has_bias


def gather_output(results):
    out = np.empty((B, L, D), dtype=np.float32)
    for b in range(B):
        for qi in range(NQ):
            acc = None
            for ki in range(NK):
                c = b * 4 + qi * 2 + ki
                part = results[c]["outT"]
                acc = part if acc is None else acc + part
            oT = acc.reshape(P, S, LQ).transpose(1, 0, 2).reshape(D, LQ)
            out[b, qi * LQ:(qi + 1) * LQ, :] = oT.T
    return out


def kernel(**inputs) -> np.ndarray:
    in_maps, has_bias = prepare_inputs(**inputs)
    nc = _get_program(has_bias)
    res = run_bass_kernel_spmd(nc, in_maps, list(range(N_CORES)))
    return gather_output(res.results)
